# revision 29
# baseline (speedup 1.0000x reference)
"""MLA (multi-head latent attention) Trainium2 kernel, 8-core SPMD.

Sharding: core c handles batch b = c//4 and heads 4*(c%4) .. 4*(c%4)+4.
Each core returns a partial [S, D] output; the host sums 4 partials per
batch + bo.

Weight absorption (host-side, free): the low-rank chains collapse to
single GEMMs against x:
  kv_up = x @ (Wd@Wu)   = x @ Au      (content K and V)
  q_c   = x @ (Wqd@Wqu) = x @ Aq
  q_r   = rope(x @ (Wqd@Wqr)) = rope(x @ Ar)
  k_r   = rope(x @ Wkr)
so the replicated kv_c / q_cmp intermediates disappear entirely.

Precision scheme (all matmuls bf16 or fp8 DoubleRow):
  - x and all absorbed weights are split hi+lo in fp8e4 (host-side);
    the four x-projections run as fp8 DoubleRow residual matmuls
    (3-pass hi*hi + lo*hi + hi*lo, or 2-pass for k_r), error ~1e-3.
    Weights prescaled by 32; qhat carries the x32 (folded into the exp
    scale), kvupT evac divides by 32, k_r folds 1/32 into cos32/sin32.
  - scores: q-hat hi/lo, k-hat hi only; 2-pass DoubleRow over the
    192-dim (content 128 + rope 64, parity-packed with zero padding)
    contraction.
  - V path (kv_up), probs, attention out bf16; Wo fp8 3-pass.
Per-core pipeline:
  P1: stream x (fp8 hi/lo): k_r rope -> khat rope hi fp8; q_r rope ->
      qhat rope hi/lo fp8; kvupT (bf16, + khat content hi fp8);
      qhatT content hi/lo fp8.  All direct from x.
  P5: scores (fp8 2-pass) -> exp -> probs bf16; AV bf16; softmax denom
      via bf16 tree-add + ones-matmul; normalize at evacuation
  P6: partial = attn_flat @ Wo (fp8 3-pass), host reduce + bo.
"""

import sys
import types

import numpy as np
import ml_dtypes

import concourse.bass as bass
import concourse.tile as tile
from concourse import mybir, bacc
from concourse.bass_utils import run_bass_kernel_spmd
from concourse.masks import make_identity

try:  # degrade gracefully if BASS_TRACE is set but the axon NTFF hook is absent
    import antenv.axon_hooks  # noqa: F401
except ImportError:
    _m = types.ModuleType("antenv.axon_hooks")
    _m.get_axon_ntff_profile_hook = lambda: None
    sys.modules["antenv.axon_hooks"] = _m

F32 = mybir.dt.float32
F8 = mybir.dt.float8e4
BF16 = mybir.dt.bfloat16
AF = mybir.ActivationFunctionType
DRM = mybir.MatmulPerfMode.DoubleRow
ALU = mybir.AluOpType
NPF8 = ml_dtypes.float8_e4m3
NPBF = ml_dtypes.bfloat16

B, S, D = 2, 2048, 2048
H, DH, DR = 16, 128, 64
DC, DQ = 512, 768
HPC = 4              # heads per core
NCORES = 8
P = 128
ND = D // P          # 16
NPD = D // 256       # 8   d-half pairs
NS = S // P          # 16
SC8 = S // 256       # 8   256-wide s chunks
KCH = S // P         # 16  key chunks
QBLK = 512
NQB = S // QBLK      # 4
WS = 32.0            # fp8 weight prescale
SCALE = float(1.0 / np.sqrt(np.float32(DH)))
ROPE_THETA = 10000.0

_NC_CACHE = {}


class _Pools:
    """Tile pools with explicit lifetimes (LIFO per (space, side) stack)."""

    def __init__(self, tc):
        self.tc = tc
        self._cms = {}
        self._order = []

    def enter(self, name, **kw):
        cm = self.tc.tile_pool(name=name, **kw)
        pool = cm.__enter__()
        self._cms[name] = cm
        self._order.append(name)
        return pool

    def exit(self, *names):
        for name in sorted(names, key=self._order.index, reverse=True):
            self._cms.pop(name).__exit__(None, None, None)
            self._order.remove(name)

    def exit_all(self):
        self.exit(*list(self._cms))


def _bcast_ap(t, n):
    """DRAM [n] vector -> AP replicated over P partitions."""
    ap = t.ap()
    return bass.AP(tensor=ap.tensor, offset=ap.offset, ap=[[0, P], [1, n]])


def _j2(ap_slice):
    """[p, (2*inner)] AP slice -> [p, 2, inner] DoubleRow half view."""
    return ap_slice.rearrange("p (j x) -> p j x", j=2)


def _emit_rope32(nc, pool, prps, bias_b, cos_ap, sin_ap, y32):
    """prps: psum [P, 2, HPC, DR] (pre-rope proj, 2 ssc batched),
    y32: sbuf [P, 2, HPC, DR] f32.  Explicit DVE/Pool split.

    Rope pairs host-permuted to deinterleaved layout: per head first 32
    dims are x1 (even original dims), last 32 are x2 (odd)."""
    pre = pool.tile([P, 2, HPC, DR], F32, tag="rope_pre")
    bb = bias_b[:][:, None, :, :].to_broadcast((P, 2, HPC, DR))
    nc.vector.tensor_add(pre[:], prps[:], bb)
    x1 = pre[:, :, :, 0:32]
    x2 = pre[:, :, :, 32:64]
    c = cos_ap[:, :, None, :].to_broadcast((P, 2, HPC, 32))
    s = sin_ap[:, :, None, :].to_broadcast((P, 2, HPC, 32))
    t1 = pool.tile([P, 2, HPC, 32], F32, tag="rope_t1")
    t2 = pool.tile([P, 2, HPC, 32], F32, tag="rope_t2")
    nc.vector.tensor_mul(t1[:], x1, c)
    nc.gpsimd.tensor_mul(t2[:], x2, s)
    nc.gpsimd.tensor_sub(y32[:, :, :, 0:32], t1[:], t2[:])
    # reuse the t1/t2 slots (tag rotation) to halve the pool footprint
    t3 = pool.tile([P, 2, HPC, 32], F32, tag="rope_t1")
    t4 = pool.tile([P, 2, HPC, 32], F32, tag="rope_t2")
    nc.gpsimd.tensor_mul(t3[:], x1, s)
    nc.vector.tensor_mul(t4[:], x2, c)
    nc.gpsimd.tensor_add(y32[:, :, :, 32:64], t3[:], t4[:])


def _build_nc():
    nc = bacc.Bacc("TRN2", target_bir_lowering=False, debug=False)

    # x hi/lo fp8, pre-tiled [s-block, p, d-pair, (half, s-in-block)]
    xh = nc.dram_tensor("xh", [SC8, P, NPD, 512], F8, kind="ExternalInput")
    xl = nc.dram_tensor("xl", [SC8, P, NPD, 512], F8, kind="ExternalInput")
    # fp8 hi/lo absorbed weights (x32), DoubleRow stationary/moving layouts
    Au8h = nc.dram_tensor("Au8h", [P, NPD, 2 * HPC * DH], F8, kind="ExternalInput")
    Au8l = nc.dram_tensor("Au8l", [P, NPD, 2 * HPC * DH], F8, kind="ExternalInput")
    Aq8h = nc.dram_tensor("Aq8h", [P, NPD, 2 * HPC * DH], F8, kind="ExternalInput")
    Aq8l = nc.dram_tensor("Aq8l", [P, NPD, 2 * HPC * DH], F8, kind="ExternalInput")
    Ar8h = nc.dram_tensor("Ar8h", [P, NPD, 2 * HPC * DR], F8, kind="ExternalInput")
    Wkr8h = nc.dram_tensor("Wkr8h", [P, NPD, 512], F8, kind="ExternalInput")
    Wo8h = nc.dram_tensor("Wo8h", [P, 2, 2, D], F8, kind="ExternalInput")
    Wo8l = nc.dram_tensor("Wo8l", [P, 2, 2, D], F8, kind="ExternalInput")
    bu = nc.dram_tensor("bu", [HPC * DH], F32, kind="ExternalInput")
    bq32 = nc.dram_tensor("bq32", [HPC * DH], F32, kind="ExternalInput")
    bqr32 = nc.dram_tensor("bqr32", [HPC * DR], F32, kind="ExternalInput")
    bkr32 = nc.dram_tensor("bkr32", [HPC * DR], F32, kind="ExternalInput")
    cosn = nc.dram_tensor("cosn", [S, DR // 2], F32, kind="ExternalInput")
    sinn = nc.dram_tensor("sinn", [S, DR // 2], F32, kind="ExternalInput")
    cos32n = nc.dram_tensor("cos32n", [S, DR // 2], F32, kind="ExternalInput")
    sin32n = nc.dram_tensor("sin32n", [S, DR // 2], F32, kind="ExternalInput")
    zeros8 = nc.dram_tensor("zeros8", [4096], F8, kind="ExternalInput")
    partial = nc.dram_tensor("partial", [S, D], BF16, kind="ExternalOutput")

    xh_b = xh.ap()
    xl_b = xl.ap()
    out_v = partial.ap().rearrange("(o p) n -> p o n", p=P)

    with tile.TileContext(nc) as tc:
        pl = _Pools(tc)
        misc = pl.enter("misc", bufs=1)

        ident = misc.tile([P, P], F32)
        ident8 = misc.tile([P, P], F8)
        identb = misc.tile([P, P], BF16)
        ones_b = misc.tile([P, 1], BF16)

        bu_s = misc.tile([P, HPC], F32)
        bq32_s = misc.tile([P, HPC], F32)
        bqr32_b = misc.tile([P, HPC, DR], F32)
        bkr32_b = misc.tile([P, HPC, DR], F32)
        cos_s = misc.tile([P, NS, DR // 2], F32)
        sin_s = misc.tile([P, NS, DR // 2], F32)
        cos32_s = misc.tile([P, NS, DR // 2], F32)
        sin32_s = misc.tile([P, NS, DR // 2], F32)

        # persistent fp8 score operands: k-hat hi only (scores are 2-pass:
        # k_hi*q_hi + k_hi*q_lo), q-hat hi/lo (parity-packed rope halves)
        khp = pl.enter("khp", bufs=1)
        khat_h = [khp.tile([P, KCH, 2 * P], F8, name=f"khh{h}") for h in range(HPC)]
        qhp = pl.enter("qhp", bufs=1)
        qhat_h = [qhp.tile([P, 2 * NQB, 512], F8, name=f"qhh{h}") for h in range(HPC)]
        qhat_l = [qhp.tile([P, 2 * NQB, 512], F8, name=f"qhl{h}") for h in range(HPC)]
        kvp = pl.enter("kvp", bufs=1)
        kvupT = kvp.tile([P, HPC, S], BF16)
        outp = pl.enter("outp", bufs=1)
        outT_h = outp.tile([P, HPC, S], F8)   # attention out^T * 64, hi
        outT_l = outp.tile([P, HPC, S], F8)   # fp8 residual
        kvn_p = pl.enter("kvn", bufs=1)
        kvupn_tiles = [kvn_p.tile([P, KCH, DH], BF16, name=f"kvupn{h}")
                       for h in range(HPC)]

        # ---- P1: absorbed projections, all direct from x ----
        p1w = pl.enter("p1w", bufs=1)
        wkr8h_s = p1w.tile([P, NPD, 512], F8)
        ar8h_s = p1w.tile([P, NPD, 2 * HPC * DR], F8)
        au8h_s = p1w.tile([P, NPD, 2 * HPC * DH], F8)
        au8l_s = p1w.tile([P, NPD, 2 * HPC * DH], F8)
        aq8h_s = p1w.tile([P, NPD, 2 * HPC * DH], F8)
        aq8l_s = p1w.tile([P, NPD, 2 * HPC * DH], F8)

        lp = pl.enter("p1loc", bufs=2, side="right")
        lp1 = pl.enter("p1st", bufs=1, side="right")
        ps_a = pl.enter("ps_a", bufs=2, space="PSUM")      # kvupT
        ps_qc = pl.enter("ps_qc", bufs=2, space="PSUM")    # q_cT
        ps_krq = pl.enter("ps_krq", bufs=1, space="PSUM")  # k-rope + q-rope
        ps_tp = pl.enter("ps_tp", bufs=1, space="PSUM")    # fp8 transposes
        ps_kvtp = pl.enter("ps_kvtp", bufs=2, space="PSUM")  # kvupn transposes

        # SP queue: x stream + the fp8 weights, ordered for chunk-0
        # readiness (k_r -> q_r -> kvupT -> q_cT).  ACT queue: small
        # tensors.  gpsimd queue: qhat/khat pad zeros.
        xch0_h = lp.tile([P, NPD, 512], F8, tag="xch_h")
        xch0_l = lp.tile([P, NPD, 512], F8, tag="xch_l")
        xch1_h = lp.tile([P, NPD, 512], F8, tag="xch_h")
        xch1_l = lp.tile([P, NPD, 512], F8, tag="xch_l")
        nc.sync.dma_start(xch0_h[:], xh_b[0])
        nc.sync.dma_start(wkr8h_s[:], Wkr8h.ap())
        nc.sync.dma_start(xch0_l[:], xl_b[0])
        nc.sync.dma_start(ar8h_s[:], Ar8h.ap())
        nc.sync.dma_start(xch1_h[:], xh_b[1])
        nc.sync.dma_start(xch1_l[:], xl_b[1])
        nc.sync.dma_start(au8h_s[:], Au8h.ap())
        nc.sync.dma_start(au8l_s[:], Au8l.ap())
        nc.sync.dma_start(aq8h_s[:], Aq8h.ap())
        nc.sync.dma_start(aq8l_s[:], Aq8l.ap())
        nc.scalar.dma_start(bkr32_b[:], _bcast_ap(bkr32, HPC * DR))
        nc.scalar.dma_start(
            cos32_s[:], cos32n.ap().rearrange("(o p) i -> p o i", p=P))
        nc.scalar.dma_start(
            sin32_s[:], sin32n.ap().rearrange("(o p) i -> p o i", p=P))
        nc.scalar.dma_start(bqr32_b[:], _bcast_ap(bqr32, HPC * DR))
        nc.scalar.dma_start(
            cos_s[:], cosn.ap().rearrange("(o p) i -> p o i", p=P))
        nc.scalar.dma_start(
            sin_s[:], sinn.ap().rearrange("(o p) i -> p o i", p=P))
        nc.scalar.dma_start(bu_s[:], bu.ap().rearrange("(o p) -> p o", p=P))
        nc.scalar.dma_start(
            bq32_s[:], bq32.ap().rearrange("(o p) -> p o", p=P))

        # identity/ones prep after the DMA dispatches
        make_identity(nc, ident)
        nc.vector.tensor_copy(ident8[:], ident[:])
        nc.vector.tensor_copy(identb[:], ident[:])
        nc.vector.memset(ones_b, float(1.0 / 64.0))

        def p1_rope(sb, xch_h, xch_l):
            eng = [nc.vector, nc.gpsimd]
            # k_r: both 128-chunks of this 256-chunk in one [P, 512] psum
            # (2-pass: (xh+xl) @ Wkr_hi)
            prps = ps_krq.tile([P, 2, HPC, DR], F32, tag='rps')
            for s2 in range(2):
                n = 0
                for xt, wt in ((xch_h, wkr8h_s), (xch_l, wkr8h_s)):
                    for o2 in range(NPD):
                        nc.tensor.matmul(
                            prps[:, s2, :, :],
                            _j2(xt[:, o2, :])[:, :, s2 * P:(s2 + 1) * P],
                            _j2(wt[:, o2, :]),
                            start=(n == 0), stop=(n == 2 * NPD - 1),
                            perf_mode=DRM)
                        n += 1
            y32 = lp1.tile([P, 2, HPC, DR], F32, tag="ky32")
            _emit_rope32(nc, lp1, prps, bkr32_b,
                         cos32_s[:, sb * 2:sb * 2 + 2, :],
                         sin32_s[:, sb * 2:sb * 2 + 2, :], y32)
            krn_h = lp1.tile([P, 2, HPC, DR], F8, tag="kr8h")
            nc.vector.tensor_copy(krn_h[:], y32[:])
            for s2 in range(2):
                ssc = sb * 2 + s2
                for j in range(2):
                    # fp8 transpose output must be element-step-2 on trn2
                    tp = ps_tp.tile([P, P, 2], F8)
                    tpv = tp[:, :, 0]
                    nc.tensor.transpose(
                        tpv, krn_h[:, s2, 2 * j:2 * j + 2, :], ident8[:])
                    # tp parts 0:64 = head 2j (parity 0), 64:128 = head
                    # 2j+1 (parity 1) -- matches khat parity packing
                    nc.scalar.copy(
                        khat_h[2 * j][0:64, ssc, P:2 * P], tp[0:64, :, 0])
                    nc.vector.tensor_copy(
                        khat_h[2 * j + 1][64:128, ssc, P:2 * P],
                        tp[64:128, :, 0])
            # q_r: 2-pass via absorbed Ar (x32 scale kept in qhat)
            prpq = ps_krq.tile([P, 2, HPC, DR], F32, tag="rps")
            for s2 in range(2):
                n = 0
                for xt, wt in ((xch_h, ar8h_s), (xch_l, ar8h_s)):
                    for o2 in range(NPD):
                        nc.tensor.matmul(
                            prpq[:, s2, :, :],
                            _j2(xt[:, o2, :])[:, :, s2 * P:(s2 + 1) * P],
                            _j2(wt[:, o2, :]),
                            start=(n == 0), stop=(n == 2 * NPD - 1),
                            perf_mode=DRM)
                        n += 1
            qy32 = lp1.tile([P, 2, HPC, DR], F32, tag="qy32")
            _emit_rope32(nc, lp1, prpq, bqr32_b,
                         cos_s[:, sb * 2:sb * 2 + 2, :],
                         sin_s[:, sb * 2:sb * 2 + 2, :], qy32)
            qrn_h = lp1.tile([P, 2, HPC, DR], F8, tag="qr8h")
            qrn_l = lp1.tile([P, 2, HPC, DR], F8, tag="qr8l")
            nc.vector.tensor_copy(qrn_h[:], qy32[:])
            nc.gpsimd.tensor_sub(qrn_l[:], qy32[:], qrn_h[:])
            for qrn8, dest in ((qrn_h, qhat_h), (qrn_l, qhat_l)):
                for s2 in range(2):
                    ssc = sb * 2 + s2
                    sc2, s_half = ssc // 2, ssc % 2
                    for j in range(2):
                        tp = ps_tp.tile([P, P, 2], F8)
                        tpv = tp[:, :, 0]
                        nc.tensor.transpose(
                            tpv, qrn8[:, s2, 2 * j:2 * j + 2, :], ident8[:])
                        d0 = _j2(dest[2 * j][0:64, sc2, :])[
                            :, 1, s_half * P:(s_half + 1) * P]
                        nc.scalar.copy(d0, tp[0:64, :, 0])
                        d1 = _j2(dest[2 * j + 1][64:128, sc2, :])[
                            :, 1, s_half * P:(s_half + 1) * P]
                        nc.vector.tensor_copy(d1, tp[64:128, :, 0])

        def p1_proj(sb, xch_h, xch_l):
            off = sb * 256
            eng = [nc.vector, nc.gpsimd]
            # kvupT: 3-pass fp8 via absorbed Au, evac /32 + bias -> bf16
            for c in range(HPC):
                psum = ps_a.tile([P, 256], F32)
                n = 0
                for wt, xt in ((au8h_s, xch_h), (au8h_s, xch_l),
                               (au8l_s, xch_h)):
                    for o2 in range(NPD):
                        nc.tensor.matmul(
                            psum[:],
                            _j2(wt[:, o2, :])[:, :, c * P:(c + 1) * P],
                            _j2(xt[:, o2, :]),
                            start=(n == 0), stop=(n == 3 * NPD - 1),
                            perf_mode=DRM)
                        n += 1
                nc.scalar.activation(
                    kvupT[:, c, off:off + 256], psum[:],
                    AF.Identity, bias=bu_s[:, c:c + 1], scale=float(1.0 / WS))
                kvv = kvupT[:, c, off:off + 256].rearrange(
                    "p (a x) -> p a x", a=2)
                eng[c % 2].tensor_copy(
                    khat_h[c][:, sb * 2:sb * 2 + 2, 0:P], kvv)
                # V-normal tiles for AV: transpose this chunk now (frees a
                # PSUM slot for double-buffered P6 evac later)
                for s2 in range(2):
                    kc = sb * 2 + s2
                    tpn = ps_kvtp.tile([P, P], BF16, tag="kvtp")
                    nc.tensor.transpose(
                        tpn[:], kvupT[:, c, kc * P:(kc + 1) * P], identb[:])
                    nc.vector.tensor_copy(kvupn_tiles[c][:, kc, :], tpn[:])
            # q_cT: 3-pass fp8 via absorbed Aq -> qhat hi/lo (x32 scale)
            for c in range(HPC):
                psum = ps_qc.tile([P, 256], F32)
                n = 0
                for wt, xt in ((aq8h_s, xch_h), (aq8h_s, xch_l),
                               (aq8l_s, xch_h)):
                    for o2 in range(NPD):
                        nc.tensor.matmul(
                            psum[:],
                            _j2(wt[:, o2, :])[:, :, c * P:(c + 1) * P],
                            _j2(xt[:, o2, :]),
                            start=(n == 0), stop=(n == 3 * NPD - 1),
                            perf_mode=DRM)
                        n += 1
                qh = _j2(qhat_h[c][:, sb, :])[:, 0, :]
                nc.scalar.activation(
                    qh, psum[:], AF.Identity, bias=bq32_s[:, c:c + 1])
                tmp = lp1.tile([P, 256], F32, tag="ctmp")
                nc.vector.tensor_sub(tmp[:], psum[:], qh)
                nc.gpsimd.tensor_scalar(
                    _j2(qhat_l[c][:, sb, :])[:, 0, :],
                    tmp[:], 1.0, bq32_s[:, c:c + 1], ALU.mult, ALU.add)

        # chunk 0/1 rope work first: it needs only Wkr/Ar, which land well
        # before the big Au/Aq transfers complete
        p1_rope(0, xch0_h, xch0_l)
        p1_rope(1, xch1_h, xch1_l)
        # anti-parity rope pad zeros (strided, small transfers); deferred
        # past the startup DMA burst -- they are only read by P5 scores
        z_ap = bass.AP(tensor=zeros8.ap().tensor, offset=0,
                       ap=[[0, 64], [1, 2048]])
        for h in range(HPC):
            za = (1 - h % 2) * 64
            nc.gpsimd.dma_start(khat_h[h][za:za + 64, :, P:2 * P], z_ap)
            for t in (qhat_h[h], qhat_l[h]):
                nc.gpsimd.dma_start(
                    t[za:za + 64, :, :].rearrange("p a (j x) -> p a j x", j=2)
                    [:, :, 1, :], z_ap)
        p1_proj(0, xch0_h, xch0_l)
        p1_proj(1, xch1_h, xch1_l)
        for sb in range(2, SC8):
            xch_h = lp.tile([P, NPD, 512], F8, tag="xch_h")
            xch_l = lp.tile([P, NPD, 512], F8, tag="xch_l")
            nc.sync.dma_start(xch_h[:], xh_b[sb])
            nc.sync.dma_start(xch_l[:], xl_b[sb])
            p1_rope(sb, xch_h, xch_l)
            p1_proj(sb, xch_h, xch_l)

        pl.exit("p1loc", "p1st", "p1w",
                "ps_a", "ps_qc", "ps_krq", "ps_tp", "ps_kvtp")

        # ---------------- P5: attention ----------------
        p6w = pl.enter("p6w", bufs=1, side="right")
        wo_sls = []
        for ncc in range(4):
            wo8h_sl = p6w.tile([P, 2, 2, 512], F8, name=f"wo8h{ncc}")
            nc.sync.dma_start(wo8h_sl[:], Wo8h.ap()[:, :, :, ncc * 512:(ncc + 1) * 512])
            wo8l_sl = p6w.tile([P, 2, 2, 512], F8, name=f"wo8l{ncc}")
            nc.sync.dma_start(wo8l_sl[:], Wo8l.ap()[:, :, :, ncc * 512:(ncc + 1) * 512])
            wo_sls.append((wo8h_sl, wo8l_sl))
        ap_ = pl.enter("attn", bufs=4)
        invp = pl.enter("invp", bufs=2)
        scps = pl.enter("scps", bufs=2, space="PSUM")
        avps = pl.enter("avps", bufs=2, space="PSUM")
        p6ps = pl.enter("p6ps", bufs=2, space="PSUM")
        lp6 = pl.enter("p6loc", bufs=3, side="right")
        OSC = float(1.0 / (64.0 * WS))
        ESCALE = float(SCALE / WS)

        # Deferred PE work queue: P6 matmul groups and per-head softmax
        # tails are emitted one-per-score-pair inside later heads' kc
        # loops, so the in-order PE queue never stalls at its head
        # waiting for an exp or a cross-engine reduction chain.
        fillers = []

        def drain_filler():
            if fillers:
                fillers.pop(0)()

        def mk_p6(s16, ncc):
            def go():
                ssl = slice(s16 * P, (s16 + 1) * P)
                wo8h_sl, wo8l_sl = wo_sls[ncc]
                psum = p6ps.tile([P, 512], F32, tag="p6", name="p6psum")
                for dc2 in range(2):
                    n = 0
                    for p2 in range(2):
                        mvh = wo8h_sl[:, p2, :, dc2 * 256:(dc2 + 1) * 256]
                        mvl = wo8l_sl[:, p2, :, dc2 * 256:(dc2 + 1) * 256]
                        sth = outT_h[:, 2 * p2:2 * p2 + 2, ssl]
                        stl = outT_l[:, 2 * p2:2 * p2 + 2, ssl]
                        for st, mv in ((sth, mvh), (stl, mvh), (sth, mvl)):
                            nc.tensor.matmul(
                                psum[:, dc2 * 256:(dc2 + 1) * 256], st, mv,
                                start=(n == 0), stop=(n == 5), perf_mode=DRM)
                            n += 1
                osb = lp6.tile([P, 512], BF16, tag="osb")
                nc.vector.tensor_scalar(
                    osb[:], psum[:], OSC, None, ALU.mult)
                nc.sync.dma_start(
                    out_v[:, s16, ncc * 512:(ncc + 1) * 512], osb[:])
            return go

        def mk_tail(h, q0, pA, pB, av):
            def go():
                smp_t = p6ps.tile([P, QBLK], F32, tag="p6", name="smp_t")
                smps = smp_t[0:1, :]
                nc.tensor.matmul(
                    smps, ones_b[:], pA[:, 0, :], start=True, stop=False)
                nc.tensor.matmul(
                    smps, ones_b[:], pB[:, 0, :], start=False, stop=True)
                inv = invp.tile([1, QBLK], F32, tag="inv")
                nc.vector.reciprocal(inv[:], smps)
                invb = invp.tile([P, QBLK], F32, tag="invb")
                nc.gpsimd.partition_broadcast(invb[:], inv[:])
                # ones held 1/64, so inv = 64/denom: outT comes out x64,
                # in fp8 normal range; P6 evac divides by 64*32*32
                o32 = invp.tile([P, QBLK], F32, tag="o32")
                nc.vector.tensor_mul(o32[:], av[:], invb[:])
                nc.gpsimd.tensor_copy(outT_h[:, h, q0:q0 + QBLK], o32[:])
                nc.gpsimd.tensor_sub(
                    outT_l[:, h, q0:q0 + QBLK], o32[:],
                    outT_h[:, h, q0:q0 + QBLK])
            return go

        for qb in range(NQB):
            q0 = qb * QBLK
            for h in range(HPC):
                kvupn = kvupn_tiles[h]
                kh_t = khat_h[h]
                qh_t, ql_t = qhat_h[h], qhat_l[h]
                pA = ap_.tile([P, KCH // 2, QBLK], BF16, tag="probsT")
                pB = ap_.tile([P, KCH // 2, QBLK], BF16, tag="probsT")
                halves = (pA, pB)
                av = avps.tile([P, QBLK], F32, tag="av", name="av")
                sps = None
                for kc in range(KCH):
                    ph, ki = halves[kc // 8], kc % 8
                    if kc % 2 == 0:
                        sps = scps.tile([P, 2 * QBLK], F32)  # 2 psum banks
                    sof = (kc % 2) * QBLK
                    st_h = _j2(kh_t[:, kc, :])
                    for qc in range(2):
                        mv_h = _j2(qh_t[:, qb * 2 + qc, :])
                        mv_l = _j2(ql_t[:, qb * 2 + qc, :])
                        trips = ((st_h, mv_h), (st_h, mv_l))
                        for i, (st, mv) in enumerate(trips):
                            nc.tensor.matmul(
                                sps[:, sof + qc * 256:sof + (qc + 1) * 256],
                                st, mv,
                                start=(i == 0), stop=(i == 1), perf_mode=DRM)
                    if kc % 2 == 1:
                        # one exp over both k-chunks (2 psum banks wide)
                        nc.scalar.activation(
                            ph[:, ki - 1:ki + 1, :], sps[:], AF.Exp,
                            scale=ESCALE)
                        # AV accumulation interleaved per k-chunk pair keeps
                        # PE fed while ACT exps the next pair.
                        for k2 in (kc - 1, kc):
                            nc.tensor.matmul(
                                av[:], kvupn[:, k2, :], ph[:, k2 % 8, :],
                                start=(k2 == 0), stop=(k2 == KCH - 1))
                        drain_filler()
                    if kc == 7 or kc == KCH - 1:
                        # in-place tree reduction of the finished half
                        # (bf16 sbuf operands -> DVE 2x/4x modes)
                        nc.gpsimd.tensor_add(
                            ph[:, 0:4, :], ph[:, 0:4, :], ph[:, 4:8, :])
                        nc.vector.tensor_add(
                            ph[:, 0:2, :], ph[:, 0:2, :], ph[:, 2:4, :])
                        nc.vector.tensor_add(
                            ph[:, 0:1, :], ph[:, 0:1, :], ph[:, 1:2, :])
                fillers.append(mk_tail(h, q0, pA, pB, av))
            if qb < NQB - 1:
                for s16 in range(qb * 4, qb * 4 + 4):
                    for ncc in range(4):
                        fillers.append(mk_p6(s16, ncc))
        while fillers:
            drain_filler()
        for s16 in range((NQB - 1) * 4, NQB * 4):
            for ncc in range(4):
                mk_p6(s16, ncc)()

        pl.exit_all()


    nc.compile()
    return nc


def _get_nc():
    if "nc" not in _NC_CACHE:
        _NC_CACHE["nc"] = _build_nc()
    return _NC_CACHE["nc"]


def _rope_tables():
    inv_freq = (1.0 / (ROPE_THETA ** (np.arange(0, DR, 2, dtype=np.float32) / DR)))
    t = np.arange(S, dtype=np.float32)
    ang = t[:, None] * inv_freq[None, :]
    return np.cos(ang).astype(np.float32), np.sin(ang).astype(np.float32)


def _hilo(W):
    hi = W.astype(NPF8)
    lo = (W - hi.astype(np.float32)).astype(NPF8)
    return hi, lo


def _st_layout(W8):
    """fp8 [R, C] -> DoubleRow d-paired layout [P, R//256, 2*C]."""
    R, C = W8.shape
    return np.ascontiguousarray(
        W8.reshape(R // 256, 2, P, C).transpose(2, 0, 1, 3).reshape(
            P, R // 256, 2 * C))


def _x_layout(x8):
    """fp8 [S, D] -> [SC8, P, NPD, 512] ([sb, p, o2, (j, si)])."""
    return np.ascontiguousarray(
        x8.T.reshape(NPD, 2, P, SC8, 256).transpose(3, 2, 0, 1, 4).reshape(
            SC8, P, NPD, 512))


def _shard_inputs(x, Wd, bd, Wu, bu, Wqd, bqd, Wqu, bqu, Wqr, bqr, Wkr, bkr, Wo):
    cosn, sinn = _rope_tables()
    perm = np.concatenate([np.arange(0, DR, 2), np.arange(1, DR, 2)])

    Wqr_h = Wqr.reshape(DQ, H, DR)[:, :, perm]
    Wkr_h = Wkr.reshape(D, H, DR)[:, :, perm]
    bqr_h = bqr.reshape(H, DR)[:, perm]
    bkr_h = bkr.reshape(H, DR)[:, perm]
    Wu_h = Wu.reshape(DC, H, DH)
    bu_h = bu.reshape(H, DH)
    Wqu_h = Wqu.reshape(DQ, H, DH)
    bqu_h = bqu.reshape(H, DH)
    Wo_h = Wo.reshape(H, DH, D)

    xhl = []
    for b in range(B):
        xhi, xlo = _hilo(x[b])
        xhl.append((_x_layout(xhi), _x_layout(xlo)))

    in_maps = []
    for c in range(NCORES):
        b = c // 4
        hs = slice((c % 4) * HPC, (c % 4) * HPC + HPC)
        # absorbed weights (exact fp32 on host)
        Au = Wd @ Wu_h[:, hs].reshape(DC, HPC * DH)          # [D, 512]
        Aq = Wqd @ Wqu_h[:, hs].reshape(DQ, HPC * DH)        # [D, 512]
        Ar = Wqd @ Wqr_h[:, hs].reshape(DQ, HPC * DR)        # [D, 256]
        bu_abs = bd @ Wu_h[:, hs].reshape(DC, HPC * DH) + bu_h[hs].reshape(-1)
        bq_abs = bqd @ Wqu_h[:, hs].reshape(DQ, HPC * DH) + bqu_h[hs].reshape(-1)
        bqr_abs = bqd @ Wqr_h[:, hs].reshape(DQ, HPC * DR) + bqr_h[hs].reshape(-1)
        Au8h, Au8l = _hilo(Au * WS)
        Aq8h, Aq8l = _hilo(Aq * WS)
        Ar8h, _ = _hilo(Ar * WS)
        Wkr8h, _ = _hilo(Wkr_h[:, hs].reshape(D, HPC * DR) * WS)
        Wo8h, Wo8l = _hilo(Wo_h[hs].reshape(HPC * DH, D) * WS)
        in_maps.append({
            "xh": xhl[b][0],
            "xl": xhl[b][1],
            "Au8h": _st_layout(Au8h),
            "Au8l": _st_layout(Au8l),
            "Aq8h": _st_layout(Aq8h),
            "Aq8l": _st_layout(Aq8l),
            "Ar8h": _st_layout(Ar8h),
            "Wkr8h": _st_layout(Wkr8h),
            "Wo8h": np.ascontiguousarray(
                Wo8h.reshape(2, 2, P, D).transpose(2, 0, 1, 3)),
            "Wo8l": np.ascontiguousarray(
                Wo8l.reshape(2, 2, P, D).transpose(2, 0, 1, 3)),
            "bu": np.ascontiguousarray(bu_abs.astype(np.float32)),
            "bq32": np.ascontiguousarray(
                (bq_abs * WS).astype(np.float32)),
            "bqr32": np.ascontiguousarray(
                (bqr_abs * WS).astype(np.float32)),
            "bkr32": np.ascontiguousarray(
                bkr_h[hs].reshape(-1)) * np.float32(WS),
            "cosn": cosn,
            "sinn": sinn,
            "cos32n": cosn / np.float32(WS),
            "sin32n": sinn / np.float32(WS),
            "zeros8": np.zeros(4096, NPF8),
        })
    return in_maps


def kernel(x, Wd, bd, Wu, bu, Wqd, bqd, Wqu, bqu, Wqr, bqr, Wkr, bkr, Wo, bo):
    args = [np.ascontiguousarray(np.asarray(a, np.float32)) for a in
            (x, Wd, bd, Wu, bu, Wqd, bqd, Wqu, bqu, Wqr, bqr, Wkr, bkr, Wo)]
    bo = np.asarray(bo, np.float32)

    nc = _get_nc()
    in_maps = _shard_inputs(*args)
    res = run_bass_kernel_spmd(nc, in_maps, core_ids=list(range(NCORES)))

    out = np.zeros((B, S, D), np.float32)
    for c in range(NCORES):
        out[c // 4] += res.results[c]["partial"]
    out += bo[None, None, :]
    return out
